# revision 16
# baseline (speedup 1.0000x reference)
"""Cross-attention block (B=16, N=4096 queries, M=77 keys, 8 heads x 64) on 8 trn2 cores.

Sharding: data-parallel over batch; each core gets 2 batches, full weights.

Per-core dataflow (matmuls bf16 in / fp32 psum), software-pipelined so the PE
always has independent work (HAM stays warm):
  x -> bf16 staging copy in DRAM (gpsimd cast DMA), xbar-transposed per chunk
  into xT [feat, tok];  qT = Wq.T @ xT.
  Per chunk g the emission order is: scores(g) -> qT(g+1) -> colsums(g) ->
  recip -> bcast/attnv/normalize(g) -> out(g), so qT(g+1) matmuls fill the
  PE while the scalar engine runs the exps of chunk g.
  Softmax denominators via indicator-matmul colsums; reciprocals broadcast
  across partitions with a K=8 indicator matmul (all on-chip, no DRAM bounce);
  aT = (v_h.T @ E_h) * recip with a both-PSUM DVE multiply.
  out = aT.T @ Wo + bo (bias added during the PSUM->SBUF copy on DVE).
"""

import numpy as np

import concourse.bass as bass
import concourse.mybir as mybir
import concourse.tile as tile
from concourse import bacc
from concourse._compat import with_exitstack
from concourse.bass_utils import run_bass_kernel_spmd
from concourse.masks import make_identity
from contextlib import ExitStack

N_CORES = 8
B, N, FEAT, CD = 16, 4096, 512, 768
M = 77          # cond tokens
H, DH = 8, 64
DA = H * DH     # 512
BP = B // N_CORES   # batches per core
TC = 512            # token chunk
NT = N // TC        # chunks per batch
NG = BP * NT        # chunks per core
SUB = TC // 128     # 128-token subtiles per chunk
KC = FEAT // 128    # x feature chunks
CC = CD // 128      # cond feature chunks
MC = DA // 128      # d_attn chunks
HPAIRS = H // 2

F32 = mybir.dt.float32
BF16 = mybir.dt.bfloat16
EXP = mybir.ActivationFunctionType.Exp


@with_exitstack
def _body(
    ctx: ExitStack, tc: tile.TileContext, x, x_bf, cond, Wq, Wk, Wv, Wo, bo, indb_d, out
):
    nc = tc.nc

    wpool = ctx.enter_context(tc.tile_pool(name="wpool", bufs=1))
    Wq_bf = wpool.tile([128, KC, DA], BF16, tag="wq")
    Wk_bf = wpool.tile([128, CC, DA], BF16, tag="wk")
    Wv_bf = wpool.tile([128, CC, DA], BF16, tag="wv")
    Wo_bf = wpool.tile([128, MC, FEAT], BF16, tag="wo")
    bo_bc = wpool.tile([128, FEAT], F32, tag="bo")
    ident = wpool.tile([128, 128], F32, tag="ident")
    # 0/1 picker: col 8 is ones; colpick[:, 8-h : 16-h] selects head h
    colpick = wpool.tile([128, 17], BF16, tag="colpick")
    # per head-pair hp: indb[2hp, hp, 0:64]=1, indb[2hp+1, hp, 64:128]=1
    indb = wpool.tile([8, HPAIRS, 128], BF16, tag="indb")

    # stage the first two x chunks before the weights so the first
    # transpose-loads (and the PE pipeline behind them) start early
    for g0 in range(2):
        nc.gpsimd.dma_start(
            out=x_bf[0, TC * g0 : TC * (g0 + 1), :],
            in_=x[0, TC * g0 : TC * (g0 + 1), :],
        )
    for k in range(KC):
        nc.gpsimd.dma_start(out=Wq_bf[:, k, :], in_=Wq[128 * k : 128 * (k + 1), :])
    for c in range(CC):
        nc.gpsimd.dma_start(out=Wk_bf[:, c, :], in_=Wk[128 * c : 128 * (c + 1), :])
        nc.gpsimd.dma_start(out=Wv_bf[:, c, :], in_=Wv[128 * c : 128 * (c + 1), :])
    for m in range(MC):
        nc.gpsimd.dma_start(out=Wo_bf[:, m, :], in_=Wo[128 * m : 128 * (m + 1), :])
    bo_bcast_ap = bass.AP(tensor=bo.tensor, offset=bo.offset, ap=[[0, 128], *bo.ap])
    nc.gpsimd.dma_start(out=bo_bc[:, :], in_=bo_bcast_ap)
    nc.sync.dma_start(out=indb[:, :, :], in_=indb_d[:, :, :])
    make_identity(nc, ident)
    nc.gpsimd.memset(colpick[:, :], 0.0)
    nc.gpsimd.memset(colpick[:, 8:9], 1.0)

    # bf16 staging copy of x (transpose-loads below need a 2-byte dtype),
    # issued chunk-by-chunk in consumption order.
    for b in range(BP):
        for t in range(2 if b == 0 else 0, NT):
            nc.gpsimd.dma_start(
                out=x_bf[b, TC * t : TC * (t + 1), :],
                in_=x[b, TC * t : TC * (t + 1), :],
            )

    bpool = ctx.enter_context(tc.tile_pool(name="bpool", bufs=2))
    tpool = ctx.enter_context(tc.tile_pool(name="tpool", bufs=4))
    qpool = ctx.enter_context(tc.tile_pool(name="qpool", bufs=3))
    epool = ctx.enter_context(tc.tile_pool(name="epool", bufs=2))
    rpool = ctx.enter_context(tc.tile_pool(name="rpool", bufs=2))
    apool = ctx.enter_context(tc.tile_pool(name="apool", bufs=2))
    opool = ctx.enter_context(tc.tile_pool(name="opool", bufs=3))

    # PSUM: psQ 2 + psS 2 + psMB (sm+bcast) 2 + psOV (po+pu) 2 = 8 banks
    psQ = ctx.enter_context(tc.tile_pool(name="psQ", bufs=2, space="PSUM"))
    psS = ctx.enter_context(tc.tile_pool(name="psS", bufs=2, space="PSUM"))
    psMB = ctx.enter_context(tc.tile_pool(name="psMB", bufs=2, space="PSUM"))
    psOV = ctx.enter_context(tc.tile_pool(name="psOV", bufs=2, space="PSUM"))

    def prep_batch(b):
        """cond[b] -> cond.T -> kT / v projections."""
        cond_sb = bpool.tile([128, CD], F32, tag="cond", name=f"cond_{b}")
        nc.sync.dma_start(out=cond_sb[:M, :], in_=cond[b, :, :])
        condT = bpool.tile([128, CC, M], BF16, tag="condT", name=f"condT_{b}")
        for c in range(CC):
            ps = psQ.tile([128, TC], F32, tag="psQ", name=f"pst_{b}_{c}")
            nc.tensor.matmul(
                ps[:128, :M],
                cond_sb[:M, 128 * c : 128 * (c + 1)],
                ident[:M, :M],
                is_transpose=True,
            )
            nc.scalar.copy(condT[:, c, :], ps[:128, :M])
        kT = bpool.tile([128, MC, M], BF16, tag="kT", name=f"kT_{b}")
        for m in range(MC):
            pk = psQ.tile([128, TC], F32, tag="psQ", name=f"psk_{b}_{m}")
            for c in range(CC):
                nc.tensor.matmul(
                    pk[:, :M],
                    Wk_bf[:, c, 128 * m : 128 * (m + 1)],
                    condT[:, c, :],
                    start=(c == 0),
                    stop=(c == CC - 1),
                )
            nc.scalar.copy(kT[:, m, :], pk[:, :M])
        pv = psQ.tile([128, TC], F32, tag="psQ", name=f"psv_{b}")
        for c in range(CC):
            nc.tensor.matmul(
                pv[:M, :],
                condT[:, c, :],
                Wv_bf[:, c, :],
                start=(c == 0),
                stop=(c == CC - 1),
            )
        v_bf = bpool.tile([128, DA], BF16, tag="v", name=f"v_{b}")
        nc.scalar.copy(v_bf[:M, :], pv[:M, :])
        return kT, v_bf

    def emit_qT(g):
        """Transpose-load x chunk g and project: qT = Wq.T @ xT."""
        b, t = divmod(g, NT)
        tok0 = t * TC
        xT = tpool.tile([128, KC, TC], BF16, tag="xT", name=f"xT_{g}")
        for k in range(KC):
            nc.sync.dma_start(
                out=xT[:, k, :],
                in_=x_bf[b, tok0 : tok0 + TC, 128 * k : 128 * (k + 1)],
                transpose=True,
            )
        qT = qpool.tile([128, MC, TC], BF16, tag="qT", name=f"qT_{g}")
        for m in range(MC):
            pq = psQ.tile([128, TC], F32, tag="psQ", name=f"psq_{g}_{m}")
            for k in range(KC):
                nc.tensor.matmul(
                    pq,
                    Wq_bf[:, k, 128 * m : 128 * (m + 1)],
                    xT[:, k, :],
                    start=(k == 0),
                    stop=(k == KC - 1),
                )
            if m % 2 == 0:
                nc.vector.tensor_copy(qT[:, m, :], pq)
            else:
                nc.scalar.copy(qT[:, m, :], pq)
        return qT

    def emit_xT(g):
        b, t = divmod(g, NT)
        tok0 = t * TC
        xT = tpool.tile([128, KC, TC], BF16, tag="xT", name=f"xTt_{g}")
        for k in range(KC):
            nc.sync.dma_start(
                out=xT[:, k, :],
                in_=x_bf[b, tok0 : tok0 + TC, 128 * k : 128 * (k + 1)],
                transpose=True,
            )
        return xT

    def emit_proj(g, xT):
        """qT = Wq.T @ xT for chunk g (xT transpose-loaded earlier)."""
        qT = qpool.tile([128, MC, TC], BF16, tag="qT", name=f"qT_{g}")
        for m in range(MC):
            pq = psQ.tile([128, TC], F32, tag="psQ", name=f"psq_{g}_{m}")
            for k in range(KC):
                nc.tensor.matmul(
                    pq,
                    Wq_bf[:, k, 128 * m : 128 * (m + 1)],
                    xT[:, k, :],
                    start=(k == 0),
                    stop=(k == KC - 1),
                )
            if m % 2 == 0:
                nc.vector.tensor_copy(qT[:, m, :], pq)
            else:
                nc.scalar.copy(qT[:, m, :], pq)
        return qT

    def emit_score_pair(g, hp, kT_s, qT_s, E):
        """Two row-tiled score matmuls (concurrent on PE) + their exps."""
        for r in range(2):
            h = 2 * hp + r
            pse = psS.tile([128, TC], F32, tag="pse", name=f"pse_{g}_{h}")
            nc.tensor.matmul(
                pse[:M, :],
                kT_s[64 * r : 64 * r + 64, hp, :],
                qT_s[64 * r : 64 * r + 64, hp, :],
                start=True,
                stop=True,
            )
            nc.scalar.activation(E[:M, h, :], pse[:M, :], func=EXP, scale=DH**-0.5)

    # software-pipeline prologue
    kT, v_bf = prep_batch(0)
    kT_next, v_next = kT, v_bf
    xT0 = emit_xT(0)
    xT1 = emit_xT(1)
    xTs = {0: xT0, 1: xT1}
    qTs = {0: emit_proj(0, xT0), 1: emit_proj(1, xT1)}
    Es = {0: epool.tile([128, H, TC], BF16, tag="E", name="E_0")}
    for hp in range(HPAIRS):
        emit_score_pair(0, hp, kT, qTs[0], Es[0])

    for g in range(NG):
        b, t = divmod(g, NT)
        tok0 = t * TC
        E = Es.pop(g)

        # chunk g+2's transpose loads (prefetch)
        if g + 2 < NG:
            xTs[g + 2] = emit_xT(g + 2)

        # per-head colsums into sm[8, tok] (exps of chunk g drained last iter)
        sm = psMB.tile([8, TC], F32, tag="mb", name=f"sm_{g}")
        for h in range(H):
            nc.tensor.matmul(
                sm,
                colpick[:M, 8 - h : 16 - h],
                E[:M, h, :],
                start=(h == 0),
                stop=(h == H - 1),
            )
        r8 = rpool.tile([8, TC], F32, tag="r8", name=f"r8_{g}")
        nc.vector.reciprocal_approx_fast(out=r8[:8, :], in_=sm[:8, :])
        r8b = rpool.tile([8, TC], BF16, tag="r8b", name=f"r8b_{g}")
        nc.vector.tensor_copy(r8b[:8, :], r8[:8, :])

        if g + 1 < NG and (g + 1) % NT == 0:
            kT_next, v_next = prep_batch(b + 1)

        # interleave: scores(g+1) pairs woven between bcast/attnv(g) pairs so
        # the PE never idles on the recip chain or the exp pacing
        aT = apool.tile([128, MC, TC], BF16, tag="aT", name=f"aT_{g}")
        if g + 1 < NG:
            Es[g + 1] = epool.tile([128, H, TC], BF16, tag="E", name=f"E_{g+1}")
        for hp in range(HPAIRS):
            if g + 1 < NG:
                emit_score_pair(g + 1, hp, kT_next, qTs[g + 1], Es[g + 1])
            psb = psMB.tile([128, TC], F32, tag="mb", name=f"psb_{g}_{hp}")
            nc.tensor.matmul(psb, indb[:8, hp, :], r8b[:8, :], start=True, stop=True)
            rs = rpool.tile([128, TC], F32, tag="rs", name=f"rs_{g}_{hp}")
            nc.vector.tensor_copy(rs[:, :], psb[:, :])
            po = psOV.tile([128, TC], F32, tag="ov", name=f"po_{g}_{hp}")
            nc.tensor.matmul(
                po[0:64, :],
                v_bf[:M, 128 * hp : 128 * hp + 64],
                E[:M, 2 * hp, :],
                start=True,
                stop=True,
            )
            nc.tensor.matmul(
                po[64:128, :],
                v_bf[:M, 128 * hp + 64 : 128 * (hp + 1)],
                E[:M, 2 * hp + 1, :],
                start=True,
                stop=True,
            )
            nc.vector.tensor_mul(aT[:, hp, :], po[:, :], rs[:, :])

        # qT(g+2): fills the PE while the aT multiplies drain on DVE
        if g + 2 < NG:
            qTs[g + 2] = emit_proj(g + 2, xTs.pop(g + 2))

        # out = aT.T @ Wo + bo (bias added during psum -> sbuf copy)
        for s in range(SUB):
            pu = psOV.tile([128, FEAT], F32, tag="ov", name=f"pu_{g}_{s}")
            for m in range(MC):
                nc.tensor.matmul(
                    pu,
                    aT[:, m, 128 * s : 128 * (s + 1)],
                    Wo_bf[:, m, :],
                    start=(m == 0),
                    stop=(m == MC - 1),
                )
            osb = opool.tile([128, FEAT], F32, tag="osb", name=f"osb_{g}_{s}")
            nc.vector.tensor_add(osb, pu, bo_bc)
            nc.sync.dma_start(
                out=out[b, tok0 + 128 * s : tok0 + 128 * (s + 1), :], in_=osb
            )

        qTs.pop(g, None)
        xTs.pop(g, None)
        if g + 1 < NG and (g + 1) % NT == 0:
            kT, v_bf = kT_next, v_next


def build():
    nc = bacc.Bacc(
        "TRN2", target_bir_lowering=False, debug=False, num_devices=N_CORES
    )
    x = nc.dram_tensor("x", [BP, N, FEAT], F32, kind="ExternalInput").ap()
    cond = nc.dram_tensor("cond", [BP, M, CD], F32, kind="ExternalInput").ap()
    Wq = nc.dram_tensor("Wq", [FEAT, DA], F32, kind="ExternalInput").ap()
    Wk = nc.dram_tensor("Wk", [CD, DA], F32, kind="ExternalInput").ap()
    Wv = nc.dram_tensor("Wv", [CD, DA], F32, kind="ExternalInput").ap()
    Wo = nc.dram_tensor("Wo", [DA, FEAT], F32, kind="ExternalInput").ap()
    bo = nc.dram_tensor("bo", [FEAT], F32, kind="ExternalInput").ap()
    indb_d = nc.dram_tensor("indb", [8, HPAIRS, 128], BF16, kind="ExternalInput").ap()
    out = nc.dram_tensor("out", [BP, N, FEAT], F32, kind="ExternalOutput").ap()
    x_bf = nc.dram_tensor("x_bf16_stage", [BP, N, FEAT], BF16).ap()
    with tile.TileContext(nc) as tc:
        _body(tc, x, x_bf, cond, Wq, Wk, Wv, Wo, bo, indb_d, out)
    nc.compile()
    return nc


_NC = None


def kernel(x, cond, Wq, Wk, Wv, Wo, bo, _trace=False):
    global _NC
    if _NC is None:
        _NC = build()
    import ml_dtypes

    indb_np = np.zeros((8, HPAIRS, 128), dtype=ml_dtypes.bfloat16)
    for hp in range(HPAIRS):
        indb_np[2 * hp, hp, 0:64] = 1
        indb_np[2 * hp + 1, hp, 64:128] = 1
    shared = {
        "Wq": np.asarray(Wq, np.float32),
        "Wk": np.asarray(Wk, np.float32),
        "Wv": np.asarray(Wv, np.float32),
        "Wo": np.asarray(Wo, np.float32),
        "bo": np.asarray(bo, np.float32),
        "indb": indb_np,
    }
    in_maps = [
        {
            "x": np.ascontiguousarray(x[BP * i : BP * (i + 1)], dtype=np.float32),
            "cond": np.ascontiguousarray(cond[BP * i : BP * (i + 1)], dtype=np.float32),
            **shared,
        }
        for i in range(N_CORES)
    ]
    res = run_bass_kernel_spmd(_NC, in_maps, list(range(N_CORES)), trace=_trace)
    out = np.concatenate([r["out"] for r in res.results], axis=0)
    if _trace:
        kernel.last_exec_time_ns = res.exec_time_ns
        kernel.last_results = res
    return out


# revision 18
# speedup vs baseline: 1.0728x; 1.0728x over previous
"""Cross-attention block (B=16, N=4096 queries, M=77 keys, 8 heads x 64) on 8 trn2 cores.

Sharding: data-parallel over batch; each core gets 2 batches, full weights.

Per-core dataflow (matmuls bf16 in / fp32 psum), software-pipelined so the PE
always has independent work (HAM stays warm):
  x -> bf16 staging copy in DRAM (gpsimd cast DMA), xbar-transposed per chunk
  into xT [feat, tok];  qT = Wq.T @ xT.
  Per chunk g the emission order is: scores(g) -> qT(g+1) -> colsums(g) ->
  recip -> bcast/attnv/normalize(g) -> out(g), so qT(g+1) matmuls fill the
  PE while the scalar engine runs the exps of chunk g.
  Softmax denominators via indicator-matmul colsums; reciprocals broadcast
  across partitions with a K=8 indicator matmul (all on-chip, no DRAM bounce);
  aT = (v_h.T @ E_h) * recip with a both-PSUM DVE multiply.
  out = aT.T @ Wo + bo (bias added during the PSUM->SBUF copy on DVE).
"""

import numpy as np

import concourse.bass as bass
import concourse.mybir as mybir
import concourse.tile as tile
from concourse import bacc
from concourse._compat import with_exitstack
from concourse.bass_utils import run_bass_kernel_spmd
from concourse.masks import make_identity
from contextlib import ExitStack

N_CORES = 8
B, N, FEAT, CD = 16, 4096, 512, 768
M = 77          # cond tokens
H, DH = 8, 64
DA = H * DH     # 512
BP = B // N_CORES   # batches per core
TC = 512            # token chunk
NT = N // TC        # chunks per batch
NG = BP * NT        # chunks per core
SUB = TC // 128     # 128-token subtiles per chunk
KC = FEAT // 128    # x feature chunks
CC = CD // 128      # cond feature chunks
MC = DA // 128      # d_attn chunks
HPAIRS = H // 2

F32 = mybir.dt.float32
BF16 = mybir.dt.bfloat16
EXP = mybir.ActivationFunctionType.Exp


@with_exitstack
def _body(
    ctx: ExitStack, tc: tile.TileContext, x, x_bf, cond, Wq, Wk, Wv, Wo, bo, indb_d, out
):
    nc = tc.nc

    wpool = ctx.enter_context(tc.tile_pool(name="wpool", bufs=1))
    Wq_bf = wpool.tile([128, KC, DA], BF16, tag="wq")
    Wk_bf = wpool.tile([128, CC, DA], BF16, tag="wk")
    Wv_bf = wpool.tile([128, CC, DA], BF16, tag="wv")
    Wo_bf = wpool.tile([128, MC, FEAT], BF16, tag="wo")
    bo_bc = wpool.tile([128, FEAT], F32, tag="bo")
    ident = wpool.tile([128, 128], F32, tag="ident")
    # 0/1 picker: col 8 is ones; colpick[:, 8-h : 16-h] selects head h
    colpick = wpool.tile([128, 17], BF16, tag="colpick")
    # per head-pair hp: indb[2hp, hp, 0:64]=1, indb[2hp+1, hp, 64:128]=1
    indb = wpool.tile([8, HPAIRS, 128], BF16, tag="indb")

    # stage the first two x chunks before the weights so the first
    # transpose-loads (and the PE pipeline behind them) start early
    for g0 in range(2):
        nc.gpsimd.dma_start(
            out=x_bf[0, TC * g0 : TC * (g0 + 1), :],
            in_=x[0, TC * g0 : TC * (g0 + 1), :],
        )
    for k in range(KC):
        nc.gpsimd.dma_start(out=Wq_bf[:, k, :], in_=Wq[128 * k : 128 * (k + 1), :])
    for c in range(CC):
        nc.gpsimd.dma_start(out=Wk_bf[:, c, :], in_=Wk[128 * c : 128 * (c + 1), :])
        nc.gpsimd.dma_start(out=Wv_bf[:, c, :], in_=Wv[128 * c : 128 * (c + 1), :])
    for m in range(MC):
        nc.gpsimd.dma_start(out=Wo_bf[:, m, :], in_=Wo[128 * m : 128 * (m + 1), :])
    bo_bcast_ap = bass.AP(tensor=bo.tensor, offset=bo.offset, ap=[[0, 128], *bo.ap])
    nc.gpsimd.dma_start(out=bo_bc[:, :], in_=bo_bcast_ap)
    nc.sync.dma_start(out=indb[:, :, :], in_=indb_d[:, :, :])
    make_identity(nc, ident)
    nc.gpsimd.memset(colpick[:, :], 0.0)
    nc.gpsimd.memset(colpick[:, 8:9], 1.0)

    # bf16 staging copy of x (transpose-loads below need a 2-byte dtype),
    # issued chunk-by-chunk in consumption order.
    for b in range(BP):
        for t in range(2 if b == 0 else 0, NT):
            nc.gpsimd.dma_start(
                out=x_bf[b, TC * t : TC * (t + 1), :],
                in_=x[b, TC * t : TC * (t + 1), :],
            )

    bpool = ctx.enter_context(tc.tile_pool(name="bpool", bufs=2))
    tpool = ctx.enter_context(tc.tile_pool(name="tpool", bufs=4))
    qpool = ctx.enter_context(tc.tile_pool(name="qpool", bufs=3))
    epool = ctx.enter_context(tc.tile_pool(name="epool", bufs=3))
    rpool = ctx.enter_context(tc.tile_pool(name="rpool", bufs=3))
    apool = ctx.enter_context(tc.tile_pool(name="apool", bufs=2))
    opool = ctx.enter_context(tc.tile_pool(name="opool", bufs=2))

    # PSUM: psQ 2 + psS 2 + psMB (sm+bcast) 2 + psOV (po+pu) 2 = 8 banks
    psQ = ctx.enter_context(tc.tile_pool(name="psQ", bufs=2, space="PSUM"))
    psS = ctx.enter_context(tc.tile_pool(name="psS", bufs=2, space="PSUM"))
    psMB = ctx.enter_context(tc.tile_pool(name="psMB", bufs=2, space="PSUM"))
    psOV = ctx.enter_context(tc.tile_pool(name="psOV", bufs=2, space="PSUM"))

    def prep_batch(b):
        """cond[b] -> cond.T -> kT / v projections."""
        cond_sb = bpool.tile([128, CD], F32, tag="cond", name=f"cond_{b}")
        nc.sync.dma_start(out=cond_sb[:M, :], in_=cond[b, :, :])
        condT = bpool.tile([128, CC, M], BF16, tag="condT", name=f"condT_{b}")
        for c in range(CC):
            ps = psQ.tile([128, TC], F32, tag="psQ", name=f"pst_{b}_{c}")
            nc.tensor.matmul(
                ps[:128, :M],
                cond_sb[:M, 128 * c : 128 * (c + 1)],
                ident[:M, :M],
                is_transpose=True,
            )
            nc.scalar.copy(condT[:, c, :], ps[:128, :M])
        kT = bpool.tile([128, MC, M], BF16, tag="kT", name=f"kT_{b}")
        for m in range(MC):
            pk = psQ.tile([128, TC], F32, tag="psQ", name=f"psk_{b}_{m}")
            for c in range(CC):
                nc.tensor.matmul(
                    pk[:, :M],
                    Wk_bf[:, c, 128 * m : 128 * (m + 1)],
                    condT[:, c, :],
                    start=(c == 0),
                    stop=(c == CC - 1),
                )
            nc.scalar.copy(kT[:, m, :], pk[:, :M])
        pv = psQ.tile([128, TC], F32, tag="psQ", name=f"psv_{b}")
        for c in range(CC):
            nc.tensor.matmul(
                pv[:M, :],
                condT[:, c, :],
                Wv_bf[:, c, :],
                start=(c == 0),
                stop=(c == CC - 1),
            )
        v_bf = bpool.tile([128, DA], BF16, tag="v", name=f"v_{b}")
        nc.scalar.copy(v_bf[:M, :], pv[:M, :])
        return kT, v_bf

    def emit_qT(g):
        """Transpose-load x chunk g and project: qT = Wq.T @ xT."""
        b, t = divmod(g, NT)
        tok0 = t * TC
        xT = tpool.tile([128, KC, TC], BF16, tag="xT", name=f"xT_{g}")
        for k in range(KC):
            nc.sync.dma_start(
                out=xT[:, k, :],
                in_=x_bf[b, tok0 : tok0 + TC, 128 * k : 128 * (k + 1)],
                transpose=True,
            )
        qT = qpool.tile([128, MC, TC], BF16, tag="qT", name=f"qT_{g}")
        for m in range(MC):
            pq = psQ.tile([128, TC], F32, tag="psQ", name=f"psq_{g}_{m}")
            for k in range(KC):
                nc.tensor.matmul(
                    pq,
                    Wq_bf[:, k, 128 * m : 128 * (m + 1)],
                    xT[:, k, :],
                    start=(k == 0),
                    stop=(k == KC - 1),
                )
            if m % 2 == 0:
                nc.vector.tensor_copy(qT[:, m, :], pq)
            else:
                nc.scalar.copy(qT[:, m, :], pq)
        return qT

    def emit_xT(g):
        b, t = divmod(g, NT)
        tok0 = t * TC
        xT = tpool.tile([128, KC, TC], BF16, tag="xT", name=f"xTt_{g}")
        for k in range(KC):
            nc.sync.dma_start(
                out=xT[:, k, :],
                in_=x_bf[b, tok0 : tok0 + TC, 128 * k : 128 * (k + 1)],
                transpose=True,
            )
        return xT

    def emit_proj(g, xT):
        """qT = Wq.T @ xT for chunk g (xT transpose-loaded earlier)."""
        qT = qpool.tile([128, MC, TC], BF16, tag="qT", name=f"qT_{g}")
        for m in range(MC):
            pq = psQ.tile([128, TC], F32, tag="psQ", name=f"psq_{g}_{m}")
            for k in range(KC):
                nc.tensor.matmul(
                    pq,
                    Wq_bf[:, k, 128 * m : 128 * (m + 1)],
                    xT[:, k, :],
                    start=(k == 0),
                    stop=(k == KC - 1),
                )
            if m % 2 == 0:
                nc.vector.tensor_copy(qT[:, m, :], pq)
            else:
                nc.scalar.copy(qT[:, m, :], pq)
        return qT

    def emit_score_pair(g, hp, kT_s, qT_s, E):
        """Two row-tiled score matmuls (concurrent on PE) + their exps."""
        for r in range(2):
            h = 2 * hp + r
            pse = psS.tile([128, TC], F32, tag="pse", name=f"pse_{g}_{h}")
            nc.tensor.matmul(
                pse[:M, :],
                kT_s[64 * r : 64 * r + 64, hp, :],
                qT_s[64 * r : 64 * r + 64, hp, :],
                start=True,
                stop=True,
            )
            nc.scalar.activation(E[:M, h, :], pse[:M, :], func=EXP, scale=DH**-0.5)

    # software-pipeline prologue
    kT, v_bf = prep_batch(0)
    kT_next, v_next = kT, v_bf
    xT0 = emit_xT(0)
    xT1 = emit_xT(1)
    xTs = {0: xT0, 1: xT1}
    qTs = {0: emit_proj(0, xT0), 1: emit_proj(1, xT1)}
    Es = {0: epool.tile([128, H, TC], BF16, tag="E", name="E_0")}
    for hp in range(HPAIRS):
        emit_score_pair(0, hp, kT, qTs[0], Es[0])

    for g in range(NG):
        b, t = divmod(g, NT)
        tok0 = t * TC
        E = Es.pop(g)

        # chunk g+2's transpose loads (prefetch)
        if g + 2 < NG:
            xTs[g + 2] = emit_xT(g + 2)

        # per-head colsums into sm[8, tok] (exps of chunk g drained last iter)
        sm = psMB.tile([8, TC], F32, tag="mb", name=f"sm_{g}")
        for h in range(H):
            nc.tensor.matmul(
                sm,
                colpick[:M, 8 - h : 16 - h],
                E[:M, h, :],
                start=(h == 0),
                stop=(h == H - 1),
            )
        r8 = rpool.tile([8, TC], F32, tag="r8", name=f"r8_{g}")
        nc.vector.reciprocal_approx_fast(out=r8[:8, :], in_=sm[:8, :])
        r8b = rpool.tile([8, TC], BF16, tag="r8b", name=f"r8b_{g}")
        nc.vector.tensor_copy(r8b[:8, :], r8[:8, :])

        if g + 1 < NG and (g + 1) % NT == 0:
            kT_next, v_next = prep_batch(b + 1)

        # interleave: scores(g+1) pairs woven between bcast/attnv(g) pairs so
        # the PE never idles on the recip chain or the exp pacing
        aT = apool.tile([128, MC, TC], BF16, tag="aT", name=f"aT_{g}")
        if g + 1 < NG:
            Es[g + 1] = epool.tile([128, H, TC], BF16, tag="E", name=f"E_{g+1}")
        for hp in range(HPAIRS):
            if g + 1 < NG:
                emit_score_pair(g + 1, hp, kT_next, qTs[g + 1], Es[g + 1])
            psb = psMB.tile([128, TC], F32, tag="mb", name=f"psb_{g}_{hp}")
            nc.tensor.matmul(psb, indb[:8, hp, :], r8b[:8, :], start=True, stop=True)
            rs = rpool.tile([128, TC], F32, tag="rs", name=f"rs_{g}_{hp}")
            nc.vector.tensor_copy(rs[:, :], psb[:, :])
            po = psOV.tile([128, TC], F32, tag="ov", name=f"po_{g}_{hp}")
            nc.tensor.matmul(
                po[0:64, :],
                v_bf[:M, 128 * hp : 128 * hp + 64],
                E[:M, 2 * hp, :],
                start=True,
                stop=True,
            )
            nc.tensor.matmul(
                po[64:128, :],
                v_bf[:M, 128 * hp + 64 : 128 * (hp + 1)],
                E[:M, 2 * hp + 1, :],
                start=True,
                stop=True,
            )
            nc.vector.tensor_mul(aT[:, hp, :], po[:, :], rs[:, :])

        # qT(g+2): fills the PE while the aT multiplies drain on DVE
        if g + 2 < NG:
            qTs[g + 2] = emit_proj(g + 2, xTs.pop(g + 2))

        # out = aT.T @ Wo + bo (bias added during psum -> sbuf copy);
        # all 4 subtiles stored with a single 1 MB DMA per chunk
        osb = opool.tile([128, SUB, FEAT], F32, tag="osb", name=f"osb_{g}")
        for s in range(SUB):
            pu = psOV.tile([128, FEAT], F32, tag="ov", name=f"pu_{g}_{s}")
            for m in range(MC):
                nc.tensor.matmul(
                    pu,
                    aT[:, m, 128 * s : 128 * (s + 1)],
                    Wo_bf[:, m, :],
                    start=(m == 0),
                    stop=(m == MC - 1),
                )
            nc.vector.tensor_add(osb[:, s, :], pu, bo_bc)
        ob = out[b, tok0 : tok0 + TC, :]
        out_ap = bass.AP(
            tensor=ob.tensor,
            offset=ob.offset,
            ap=[[FEAT, 128], [128 * FEAT, SUB], [1, FEAT]],
        )
        nc.sync.dma_start(out=out_ap, in_=osb[:, :, :])

        qTs.pop(g, None)
        xTs.pop(g, None)
        if g + 1 < NG and (g + 1) % NT == 0:
            kT, v_bf = kT_next, v_next


def build():
    nc = bacc.Bacc(
        "TRN2", target_bir_lowering=False, debug=False, num_devices=N_CORES
    )
    x = nc.dram_tensor("x", [BP, N, FEAT], F32, kind="ExternalInput").ap()
    cond = nc.dram_tensor("cond", [BP, M, CD], F32, kind="ExternalInput").ap()
    Wq = nc.dram_tensor("Wq", [FEAT, DA], F32, kind="ExternalInput").ap()
    Wk = nc.dram_tensor("Wk", [CD, DA], F32, kind="ExternalInput").ap()
    Wv = nc.dram_tensor("Wv", [CD, DA], F32, kind="ExternalInput").ap()
    Wo = nc.dram_tensor("Wo", [DA, FEAT], F32, kind="ExternalInput").ap()
    bo = nc.dram_tensor("bo", [FEAT], F32, kind="ExternalInput").ap()
    indb_d = nc.dram_tensor("indb", [8, HPAIRS, 128], BF16, kind="ExternalInput").ap()
    out = nc.dram_tensor("out", [BP, N, FEAT], F32, kind="ExternalOutput").ap()
    x_bf = nc.dram_tensor("x_bf16_stage", [BP, N, FEAT], BF16).ap()
    with tile.TileContext(nc) as tc:
        _body(tc, x, x_bf, cond, Wq, Wk, Wv, Wo, bo, indb_d, out)
    nc.compile()
    return nc


_NC = None


def kernel(x, cond, Wq, Wk, Wv, Wo, bo, _trace=False):
    global _NC
    if _NC is None:
        _NC = build()
    import ml_dtypes

    indb_np = np.zeros((8, HPAIRS, 128), dtype=ml_dtypes.bfloat16)
    for hp in range(HPAIRS):
        indb_np[2 * hp, hp, 0:64] = 1
        indb_np[2 * hp + 1, hp, 64:128] = 1
    shared = {
        "Wq": np.asarray(Wq, np.float32),
        "Wk": np.asarray(Wk, np.float32),
        "Wv": np.asarray(Wv, np.float32),
        "Wo": np.asarray(Wo, np.float32),
        "bo": np.asarray(bo, np.float32),
        "indb": indb_np,
    }
    in_maps = [
        {
            "x": np.ascontiguousarray(x[BP * i : BP * (i + 1)], dtype=np.float32),
            "cond": np.ascontiguousarray(cond[BP * i : BP * (i + 1)], dtype=np.float32),
            **shared,
        }
        for i in range(N_CORES)
    ]
    res = run_bass_kernel_spmd(_NC, in_maps, list(range(N_CORES)), trace=_trace)
    out = np.concatenate([r["out"] for r in res.results], axis=0)
    if _trace:
        kernel.last_exec_time_ns = res.exec_time_ns
        kernel.last_results = res
    return out
